# revision 44
# baseline (speedup 1.0000x reference)
"""Trainium2 Bass kernel for nn_AdaptiveSoftmax (8 NeuronCores, data-parallel).

Adaptive softmax loss: head [2002 vocab] + two low-rank tail clusters
(8000 @ rank-256, 40257 @ rank-64).  Per row: out = logprob at target,
loss = mean(-out).

Strategy (per core, 512 rows, bf16 matmul / f32 accumulation):
  - Host precomputes transposed, bf16, log2e-scaled weight layouts, the
    gathered target weight rows (so no on-device indexing), and cluster
    masks.  Rows are permuted so cluster-1 (tail0) rows land in row-tile 0
    of each core, collapsing tail0 work 4x.
  - Device computes all logits with TensorE (tail1's K=64 matmuls are
    row-packed two-at-a-time via tile_position) into a single [128,4096]
    PSUM ring (all 8 banks, Tile subtile deps), drained by two parallel
    exp lanes computing the log-sum-exp:
      ACT lane:  activation(Exp, scale=ln2, accum_out) in-place on PSUM,
                 double-buffered 1536-col regions
      DVE lane:  Schraudolph fast-exp2 (tensor_scalar f32->int32 convert,
                 bitcast f32, pair-batched reduce) on a 1024-col region --
                 ~3% elementwise error, zero-mean, so sums are good to 1e-3
  - Gathered logit z_t = <x, W[t]> via scalar_tensor_tensor accum dots.
  - out = (zh - lse_h) + m0*(z0 - lse0) + m1*(z1 - lse1)
  - per-core loss partials summed on the host during unshard (a device
    AllReduce of the scalar measured ~38us and was dropped).
"""

import sys

sys.path.insert(0, "/opt/trn_rl_repo")

import numpy as np
import ml_dtypes

import concourse.bass as bass
import concourse.tile as tile
from concourse import bacc, mybir
from concourse.bass_utils import run_bass_kernel_spmd

BF16 = ml_dtypes.bfloat16

# ---- problem constants (hardcoded; harness supplies matching shapes) ----
N = 4096
NDIM = 1024
NTOK = 50257
C0, C1 = 2000, 10000
HS = 2002  # head size
D0, D1 = 256, 64
SZ0 = C1 - C0          # 8000
SZ1 = NTOK - C1        # 40257
NC = 8                 # cores
R = N // NC            # 512 rows per core
RT = R // 128          # 4 row tiles
H1 = (SZ1 + 1) // 2    # 20129 stacked-half width
PADCOLS = 2 * H1 - SZ1 # 1 pad column in half B

LOG2E = float(np.log2(np.e))
LN2 = float(np.log(2.0))
SC23 = float(1 << 23)
MAGIC_C = 482583.0
MAGIC = float(127 * (1 << 23) - MAGIC_C)

_OPTS = dict(
    use_dve_lane=True,
    group_rows=True,
    chunk_cols=2048,      # psum chunk free size (multiple of 512 + remainder)
    psum_bufs=2,
    use_collective=False,
    act_cost=lambda n: (n + 690.0) / 1.2,               # ns per ACT chunk (meas)
    dve_cost=lambda n: 2.0 * (n + 150.0) / 0.96,        # ns per DVE chunk (meas)
)

f32 = mybir.dt.float32
bf16 = mybir.dt.bfloat16
i32 = mybir.dt.int32
AX = mybir.AxisListType
OP = mybir.AluOpType
AF = mybir.ActivationFunctionType


# --------------------------------------------------------------------------
# host-side prep
# --------------------------------------------------------------------------

def _host_prep(inputs, targets, head_W, emb0, lin0, emb1, lin1, opts):
    x = np.asarray(inputs, np.float32)
    t = np.asarray(targets).astype(np.int64)
    head_W = np.asarray(head_W, np.float32)
    emb0 = np.asarray(emb0, np.float32)
    lin0 = np.asarray(lin0, np.float32)
    emb1 = np.asarray(emb1, np.float32)
    lin1 = np.asarray(lin1, np.float32)

    cluster = np.zeros(N, np.int64)
    cluster[(t >= C0) & (t < C1)] = 1
    cluster[t >= C1] = 2

    if opts["group_rows"]:
        # deal rows of each cluster round-robin across cores, then order each
        # core's rows cluster1-first so tail0 rows pack into the first tiles
        core_rows = [[] for _ in range(NC)]
        for cl in (1, 2, 0):
            idx = np.nonzero(cluster == cl)[0]
            for j, row in enumerate(idx):
                core_rows[j % NC].append(row)
        order = []
        for c in range(NC):
            rows = core_rows[c]
            rows.sort(key=lambda r: (0 if cluster[r] == 1 else 1))
            order.extend(rows)
        perm = np.array(order, np.int64)
        n1_max = max(int((cluster[perm[c * R:(c + 1) * R]] == 1).sum())
                     for c in range(NC))
        t0_nrt = max(1, -(-n1_max // 128))
    else:
        perm = np.arange(N, dtype=np.int64)
        t0_nrt = RT

    xp = x[perm]
    tp = t[perm]
    head_targets = np.where(tp >= C1, C0 + 1, np.where(tp >= C0, C0, tp))
    rel0 = np.clip(tp - C0, 0, SZ0 - 1)
    rel1 = np.clip(tp - C1, 0, SZ1 - 1)
    m0 = ((tp >= C0) & (tp < C1)).astype(np.float32)
    m1 = (tp >= C1).astype(np.float32)

    if opts["group_rows"]:
        # every tail0 row must live in the first t0_nrt tiles of its core
        for c in range(NC):
            mm = m0[c * R:(c + 1) * R]
            assert mm[t0_nrt * 128:].sum() == 0, "tail0 row outside tail0 tiles"

    Wg = head_W[head_targets]          # [N,1024]
    e0g = emb0[rel0]                   # [N,256]
    e1g = emb1[rel1]                   # [N,64]

    WT = np.ascontiguousarray(head_W.T) * LOG2E        # [1024,2002]
    e0T = np.ascontiguousarray(emb0.T) * LOG2E         # [256,8000]
    e1T = np.ascontiguousarray(emb1.T) * LOG2E         # [64,40257]
    E1 = np.zeros((128, H1), np.float32)
    E1[:64] = e1T[:, :H1]
    E1[64:, : SZ1 - H1] = e1T[:, H1:]

    def b(a):
        return np.ascontiguousarray(a).astype(BF16)

    def cols(a3):
        # [G,128,F] -> [128, G*F] keeping each [128,F] block column-contig
        return np.concatenate([a3[g] for g in range(a3.shape[0])], axis=1)

    # consolidated column-packed blocks (one DMA each, sliced on SBUF side)
    pk_wt = np.concatenate(
        [cols(WT.reshape(8, 128, HS)), cols(lin0.reshape(8, 128, D0))], axis=1)
    pk_e0 = cols(e0T.reshape(2, 128, SZ0))
    e1s = b(E1)
    ident = np.eye(128, dtype=np.float32)

    in_maps = []
    for c in range(NC):
        s = slice(c * R, (c + 1) * R)
        xc = xp[s]
        xT = np.ascontiguousarray(xc.T)            # [1024,512]
        pk_x = np.concatenate(
            [cols(xT.reshape(8, 128, R)), cols(lin1.reshape(8, 128, D1))],
            axis=1)
        pk_g = np.concatenate(
            [cols(xc.reshape(RT, 128, NDIM)),
             cols(Wg[s].reshape(RT, 128, NDIM)),
             cols(e0g[s].reshape(RT, 128, D0)),
             cols(e1g[s].reshape(RT, 128, D1))], axis=1)
        in_maps.append({
            "pkx": b(pk_x),             # xt 8x512 | lin1 8x64
            "e1s": e1s,                 # stacked tail1 (log2e-scaled)
            "pkw": b(pk_wt),            # wt 8x2002 | lin0 8x256  (replicated)
            "pke0": b(pk_e0),           # e0t 2x8000              (replicated)
            "pkg": b(pk_g),             # x4 | wg | e0g | e1g  (4x each)
            "msk": np.ascontiguousarray(
                np.stack([m0[s], m1[s]], axis=-1).reshape(RT, 128, 2)
            ).astype(np.float32),
            "ident": ident,
        })
    return in_maps, perm, t0_nrt


# --------------------------------------------------------------------------
# chunk planning
# --------------------------------------------------------------------------

def _plan_cluster_chunks(mms, chunk_cols):
    """Group MM specs into psum chunks.  HW constraint: every matmul output
    must START on a PSUM bank boundary (multiple of 512 f32), so slots are
    512-aligned; a partial-width MM (n<512) ends its chunk so the chunk's
    written region stays contiguous from offset 0."""
    chunks = []
    cur, off = [], 0
    for mm in mms:
        n = mm["n"]
        if off + n > chunk_cols:
            chunks.append(cur)
            cur, off = [], 0
        cur.append((off, mm))
        if n < 512:
            chunks.append(cur)
            cur, off = [], 0
        else:
            off += 512
    if cur:
        chunks.append(cur)
    return chunks


def _col_splits(total, step=512):
    out = []
    c = 0
    while c < total:
        n = min(step, total - c)
        out.append((c, n))
        c += n
    return out


# --------------------------------------------------------------------------
# device program
# --------------------------------------------------------------------------

def _build(t0_nrt, opts):
    nc = bacc.Bacc("TRN2", target_bir_lowering=False, debug=False, num_devices=NC)

    PKX = 8 * R + 8 * D1
    PKW = 8 * HS + 8 * D0
    PKE0 = 2 * SZ0
    PKG = RT * (NDIM + NDIM + D0 + D1)
    d_pkx = nc.dram_tensor("pkx", [128, PKX], bf16, kind="ExternalInput")
    d_e1s = nc.dram_tensor("e1s", [128, H1], bf16, kind="ExternalInput")
    d_pkw = nc.dram_tensor("pkw", [128, PKW], bf16, kind="ExternalInput")
    d_pke0 = nc.dram_tensor("pke0", [128, PKE0], bf16, kind="ExternalInput")
    d_pkg = nc.dram_tensor("pkg", [128, PKG], bf16, kind="ExternalInput")
    d_msk = nc.dram_tensor("msk", [RT, 128, 2], f32, kind="ExternalInput")
    d_ident = nc.dram_tensor("ident", [128, 128], f32, kind="ExternalInput")

    d_out = nc.dram_tensor("out", [R], f32, kind="ExternalOutput")
    d_loss = nc.dram_tensor("loss", [1, 1], f32, kind="ExternalOutput")

    CH = opts["chunk_cols"]
    act_cost, dve_cost = opts["act_cost"], opts["dve_cost"]

    with tile.TileContext(nc) as tc:
        with (
            tc.tile_pool(name="w", bufs=1) as w,        # persistent
            tc.tile_pool(name="dyn", bufs=3) as dyn,    # rotating scratch
            tc.tile_pool(name="ps", bufs=1, space="PSUM") as ps,
            tc.tile_pool(name="dram", bufs=1, space="DRAM") as dram,
        ):
            # ONE psum tile spanning all 8 banks; every PSUM user takes
            # 512-aligned regions of it (Tile subtile deps handle sync)
            ring = ps.tile([128, 4096], f32, tag="ring", name="ring")
            _pj = [0]

            def pstile(name):
                # projection scratch: alternate two 512-col ring regions
                base = (_pj[0] % 2) * 512
                _pj[0] += 1
                return ring[:, base: base + 512]
            # ---------------- input DMA (consolidated, need-ordered) -------
            pkx_t = w.tile([128, PKX], bf16, name="pkx", tag="pkx")
            # split so h1T's K-accumulation starts as soon as the first xt
            # tiles land (subtile deps) instead of after the whole 1.2MB
            nc.sync.dma_start(pkx_t[:, 8 * R:], d_pkx[:, 8 * R:])  # lin1 first
            for j in range(0, 8 * R, 2 * R):
                nc.sync.dma_start(pkx_t[:, j: j + 2 * R],
                                  d_pkx[:, j: j + 2 * R])
            xt_t = [pkx_t[:, k * R:(k + 1) * R] for k in range(8)]
            lin1_t = [pkx_t[:, 8 * R + k * D1: 8 * R + (k + 1) * D1]
                      for k in range(8)]

            e1_t = w.tile([128, H1], bf16, name="e1t", tag="e1t")
            E1SPLIT = 4
            _e1step = ((H1 + E1SPLIT - 1) // E1SPLIT + 511) // 512 * 512
            for j in range(0, H1, _e1step):
                jn = min(_e1step, H1 - j)
                nc.sync.dma_start(e1_t[:, j: j + jn], d_e1s[:, j: j + jn])

            pkw_t = w.tile([128, PKW], bf16, name="pkw", tag="pkw")
            nc.sync.dma_start(pkw_t[:], d_pkw[:, :])
            wt_t = [pkw_t[:, k * HS:(k + 1) * HS] for k in range(8)]
            lin0_t = [pkw_t[:, 8 * HS + k * D0: 8 * HS + (k + 1) * D0]
                      for k in range(8)]

            pke0_t = w.tile([128, PKE0], bf16, name="pke0", tag="pke0")
            nc.sync.dma_start(pke0_t[:], d_pke0[:, :])
            e0t_t = [pke0_t[:, k * SZ0:(k + 1) * SZ0] for k in range(2)]

            pkg_t = w.tile([128, PKG], bf16, name="pkg", tag="pkg")
            nc.sync.dma_start(pkg_t[:], d_pkg[:, :])
            _o = 0
            x_t = [pkg_t[:, _o + r * NDIM: _o + (r + 1) * NDIM]
                   for r in range(RT)]
            _o += RT * NDIM
            wg_t = [pkg_t[:, _o + r * NDIM: _o + (r + 1) * NDIM]
                    for r in range(RT)]
            _o += RT * NDIM
            e0g_t = [pkg_t[:, _o + r * D0: _o + (r + 1) * D0]
                     for r in range(RT)]
            _o += RT * D0
            e1g_t = [pkg_t[:, _o + r * D1: _o + (r + 1) * D1]
                     for r in range(RT)]
            msk_t = [w.tile([128, 2], f32, name=f"msk{r}", tag=f"msk{r}")
                     for r in range(RT)]
            for r in range(RT):
                nc.sync.dma_start(msk_t[r][:], d_msk[r])
            ident_t = w.tile([128, 128], f32, name="ident", tag="ident")
            nc.sync.dma_start(ident_t[:], d_ident[:, :])

            # ---------------- projections ---------------------------------
            # only h1T (which gates tail1) runs up-front; h0T and the per-rt
            # h0/h1 z-dot projections are woven into the main loop as thunks
            h1T_sb = w.tile([128, R], bf16, name="h1T", tag="h1T")
            pt = pstile("pj1")
            for h in range(2):
                for k in range(8):
                    nc.tensor.matmul(
                        pt[h * 64:(h + 1) * 64, :R], lin1_t[k][:], xt_t[k][:],
                        start=(k == 0), stop=(k == 7),
                        tile_position=(0, h * 64),
                    )
            nc.vector.tensor_copy(h1T_sb[:], pt[:, :R])

            h0T_sb = [w.tile([128, R], bf16, name=f"h0T{m}", tag=f"h0T{m}")
                      for m in range(2)]
            h0_sb = [w.tile([128, D0], bf16, name=f"h0_{r}", tag=f"h0_{r}")
                     for r in range(RT)]
            h1_sb = [w.tile([128, D1], bf16, name=f"h1_{r}", tag=f"h1_{r}")
                     for r in range(RT)]
            zh = [w.tile([128, 1], f32, name=f"zh{r}", tag=f"zh{r}")
                  for r in range(RT)]
            z0 = [w.tile([128, 1], f32, name=f"z0{r}", tag=f"z0{r}")
                  for r in range(RT)]
            z1 = [w.tile([128, 1], f32, name=f"z1{r}", tag=f"z1{r}")
                  for r in range(RT)]

            # ---------------- chunk plans ----------------------------------
            # MM spec: dict(kind, n, ...); kinds: head, t0, t1
            def head_mms(r):
                return [dict(kind="head", r=r, c0=c, n=n)
                        for c, n in _col_splits(HS)]

            def t0_mms(r):
                return [dict(kind="t0", r=r, c0=c, n=n)
                        for c, n in _col_splits(SZ0)]

            def t1_mms(r):
                out = []
                for c, n in _col_splits(H1):
                    out.append(dict(kind="t1", r=r, h=0, c0=c, n=n))
                    out.append(dict(kind="t1", r=r, h=1, c0=c, n=n))
                return out

            # ------------- unit planning (ring regions + lanes) -------------
            # PSUM = one [128,4096] ring tile (all 8 banks).  Fixed regions:
            # A0=[0:1536], A1=[1536:3072] double-buffer the ACT lane;
            # D0=[3072:4096] feeds the DVE lane, whose q-reduce overlaps the
            # next fill.  Tile's subtile deps give region-level sync.
            A_COLS, D_COLS = 1536, 1024

            def plan_units(mms, lane_time, is_head, act_after=1.0):
                units = []
                i = 0
                while i < len(mms):
                    if not opts["use_dve_lane"] or i >= act_after * len(mms):
                        # tail of the emission stream: DVE still owes flush
                        # reduces + the combine chain there, so keep it free
                        lane = "act"
                    else:
                        lane = ("act" if lane_time["act"] <= lane_time["dve"]
                                else "dve")
                    want = 512 if is_head else (A_COLS if lane == "act"
                                                else D_COLS)
                    group, used, slots = [], 0, 0
                    while i < len(mms) and slots * 512 + mms[i]["n"] <= want:
                        mm = mms[i]
                        group.append((slots * 512, mm))
                        used = slots * 512 + mm["n"]
                        slots += 1
                        i += 1
                        if mm["n"] < 512:
                            break
                    if lane == "act":
                        lane_time["act"] += act_cost(used)
                    else:
                        lane_time["dve"] += dve_cost(used)
                    units.append(dict(lane=lane, group=group, used=used))
                return units

            lane_time = {"act": 0.0, "dve": 0.0}
            plans = {}
            for r in range(RT):
                t1_act_after = 0.6 if r == RT - 1 else 1.0
                plans[(r, "t1")] = plan_units(t1_mms(r), lane_time, False,
                                              act_after=t1_act_after)
                plans[(r, "head")] = plan_units(head_mms(r), lane_time, True)
                if r < t0_nrt:
                    # t0 units weave at the very end of the stream where the
                    # DVE lane is the long pole -- keep them on ACT
                    plans[(r, "t0")] = plan_units(t0_mms(r), lane_time, False,
                                                  act_after=0.0)

            # emission order: tail1 from the start; everything else woven in
            t1_seq = [((r, "t1"), ci) for r in range(RT)
                      for ci in range(len(plans[(r, "t1")]))]
            extras = [("h0T", 0), ("h0T", 1)]
            extras += [((r, "head"), ci) for r in range(RT)
                       for ci in range(len(plans[(r, "head")]))]
            extras += [((r, "t0"), ci) for r in range(t0_nrt)
                       for ci in range(len(plans[(r, "t0")]))]
            extras += [("hz", r) for r in range(RT)]
            # extras need their weights: pkw ~30us (~pos 16), pke0 ~41us,
            # pkg ~49us.  Weave every 2nd t1 unit from pos 16, then every
            # unit so all extras land before the t1 stream ends.
            npos = len(t1_seq)
            positions = list(range(16, min(48, npos), 2))
            positions += list(range(max(positions[-1] + 1, 48) if positions
                                    else 48, npos))
            pos_iter = iter(positions)
            nxt = next(pos_iter, None)
            seq = []
            ei = 0
            for idx, item in enumerate(t1_seq):
                seq.append(item)
                while nxt is not None and idx == nxt and ei < len(extras):
                    seq.append(extras[ei])
                    ei += 1
                    nxt = next(pos_iter, None)
            seq.extend(extras[ei:])

            # accumulator slot count: one per act unit + one per dve flush.
            # Only ONE dve q-group may be open at a time (q-ring reuse), so
            # a key change flushes the previous owner's group -> replay seq.
            nslots = {key: 0 for key in plans}
            _owner, _pend = [None], [0]
            for key, ci in seq:
                if key in ("h0T", "hz"):
                    continue
                u = plans[key][ci]
                if u["lane"] == "act":
                    nslots[key] += 1
                else:
                    if _owner[0] is not None and _owner[0] != key and _pend[0]:
                        nslots[_owner[0]] += 1
                        _pend[0] = 0
                    _owner[0] = key
                    _pend[0] += 1
                    if _pend[0] == 2:
                        nslots[key] += 1
                        _pend[0] = 0
            if _pend[0]:
                nslots[_owner[0]] += 1
            sums = {}
            for key in plans:
                r, cl = key
                sums[key] = w.tile([128, max(nslots[key], 1)], f32,
                                   name=f"s_{cl}{r}", tag=f"s_{cl}{r}")

            qring = w.tile([128, 4096], i32, name="qring", tag="qring")

            state = dict(a_flip=0, q_flip=0, acc_ci={k: 0 for k in plans},
                         pend={k: [] for k in plans}, owner=None)

            def emit_mms(base, group, r):
                for off, mm in group:
                    dst = ring[:, base + off: base + off + mm["n"]]
                    if mm["kind"] == "head":
                        for k in range(8):
                            nc.tensor.matmul(
                                dst, xt_t[k][:, r * 128:(r + 1) * 128],
                                wt_t[k][:, mm["c0"]: mm["c0"] + mm["n"]],
                                start=(k == 0), stop=(k == 7),
                            )
                    elif mm["kind"] == "t0":
                        for k in range(2):
                            nc.tensor.matmul(
                                dst, h0T_sb[k][:, r * 128:(r + 1) * 128],
                                e0t_t[k][:, mm["c0"]: mm["c0"] + mm["n"]],
                                start=(k == 0), stop=(k == 1),
                            )
                    else:
                        h = mm["h"]
                        nc.tensor.matmul(
                            dst,
                            h1T_sb[h * 64:(h + 1) * 64, r * 128:(r + 1) * 128],
                            e1_t[h * 64:(h + 1) * 64,
                                 mm["c0"]: mm["c0"] + mm["n"]],
                            start=True, stop=True,
                            tile_position=(h * 64, 0),
                        )

            def flush_dve(key):
                pend = state["pend"][key]
                if not pend:
                    return
                base = pend[0][0]
                total = sum(p[1] for p in pend)
                ci = state["acc_ci"][key]
                nc.vector.tensor_reduce(
                    sums[key][:, ci: ci + 1],
                    qring[:, base: base + total].bitcast(f32),
                    AX.X, OP.add,
                )
                state["acc_ci"][key] = ci + 1
                state["pend"][key] = []

            # pad column in half B contributes exp(0)'s lane-dependent value
            # to every t1 sum (ACT: 1.0, DVE: bitcast(round(MAGIC)))
            dve_pad = float(
                np.array([int(round(MAGIC))], np.int32).view(np.float32)[0]
            )
            stot = {}
            for key in plans:
                r, cl = key
                stot[key] = w.tile([128, 1], f32, name=f"st_{cl}{r}",
                                   tag=f"st_{cl}{r}")
            remaining = {key: len(plans[key]) for key in plans}

            def key_done(key):
                # eager per-key total: flush leftovers + reduce sums + pad fix
                if state["owner"] == key:
                    flush_dve(key)
                    state["owner"] = None
                r, cl = key
                nc.vector.tensor_reduce(stot[key][:], sums[key][:], AX.X,
                                        OP.add)
                if cl == "t1":
                    pad_lane = plans[key][-1]["lane"]
                    pad = (1.0 if pad_lane == "act" else dve_pad) * PADCOLS
                    nc.vector.tensor_scalar_add(stot[key][:], stot[key][:],
                                                -pad)

            def emit_unit(key, unit):
                r, cl = key
                used = unit["used"]
                if unit["lane"] == "act":
                    base = state["a_flip"] * A_COLS
                    state["a_flip"] ^= 1
                    emit_mms(base, unit["group"], r)
                    src = ring[:, base: base + used]
                    ci = state["acc_ci"][key]
                    state["acc_ci"][key] = ci + 1
                    state["last_exp"] = nc.scalar.activation(
                        src, src, AF.Exp, scale=LN2,
                        accum_out=sums[key][:, ci: ci + 1])
                else:
                    base = 3072
                    emit_mms(base, unit["group"], r)
                    if state["owner"] is not None and state["owner"] != key:
                        flush_dve(state["owner"])
                    state["owner"] = key
                    pend = state["pend"][key]
                    if not pend:
                        qbase = state["q_flip"] * 2048
                        state["q_flip"] ^= 1
                    else:
                        qbase = pend[0][0] + pend[0][1]
                    nc.vector.tensor_scalar(
                        qring[:, qbase: qbase + used],
                        ring[:, base: base + used],
                        SC23, MAGIC, OP.mult, OP.add,
                    )
                    pend.append((qbase, used))
                    if len(pend) == 2:
                        flush_dve(key)
                remaining[key] -= 1
                if remaining[key] == 0:
                    key_done(key)

            def emit_h0T(m):
                base = state["a_flip"] * A_COLS
                state["a_flip"] ^= 1
                pt = ring[:, base: base + R]
                for k in range(8):
                    nc.tensor.matmul(
                        pt, lin0_t[k][:, m * 128:(m + 1) * 128],
                        xt_t[k][:], start=(k == 0), stop=(k == 7),
                    )
                nc.vector.tensor_copy(h0T_sb[m][:], pt)

            def emit_hz(r):
                # h0/h1 projections + the three gathered-logit dots for rt r
                base = state["a_flip"] * A_COLS
                state["a_flip"] ^= 1
                pt = ring[:, base: base + D0]
                for k in range(8):
                    nc.tensor.matmul(
                        pt, xt_t[k][:, r * 128:(r + 1) * 128],
                        lin0_t[k][:], start=(k == 0), stop=(k == 7),
                    )
                nc.vector.tensor_copy(h0_sb[r][:], pt)
                pt2 = ring[:, base + 512: base + 512 + D1]
                for k in range(8):
                    nc.tensor.matmul(
                        pt2, xt_t[k][:, r * 128:(r + 1) * 128],
                        lin1_t[k][:], start=(k == 0), stop=(k == 7),
                    )
                nc.vector.tensor_copy(h1_sb[r][:], pt2)
                s = dyn.tile([128, NDIM], bf16, tag="ttr", name="ttr")
                nc.vector.scalar_tensor_tensor(
                    s[:, :NDIM], x_t[r][:], 1.0, wg_t[r][:],
                    OP.mult, OP.mult, accum_out=zh[r][:],
                )
                s = dyn.tile([128, NDIM], bf16, tag="ttr", name="ttr")
                nc.vector.scalar_tensor_tensor(
                    s[:, :D0], h0_sb[r][:], 1.0, e0g_t[r][:],
                    OP.mult, OP.mult, accum_out=z0[r][:],
                )
                s = dyn.tile([128, NDIM], bf16, tag="ttr", name="ttr")
                nc.vector.scalar_tensor_tensor(
                    s[:, :D1], h1_sb[r][:], 1.0, e1g_t[r][:],
                    OP.mult, OP.mult, accum_out=z1[r][:],
                )

            for key, ci in seq:
                if key == "h0T":
                    emit_h0T(ci)
                elif key == "hz":
                    emit_hz(ci)
                else:
                    emit_unit(key, plans[key][ci])

            # ------------- epilogue: ln + combine + outputs ----------------
            outsb = w.tile([128, RT], f32, name="outsb", tag="outsb")
            lse = {}
            for key in plans:
                r, cl = key
                lse[key] = w.tile([128, 1], f32, name=f"lse_{cl}{r}",
                                  tag=f"lse_{cl}{r}")
            # pin the Ln block after the last Exp: eager stot frees Ln deps
            # mid-run and the scheduler would interleave Ln into ACT idle
            # gaps, paying a ~2.6us ACT table-set round-trip each time
            for key in plans:
                ln = nc.scalar.activation(lse[key][:], stot[key][:], AF.Ln)
                if state.get("last_exp") is not None:
                    tile.add_dep_helper(ln.ins, state["last_exp"].ins,
                                        sync=True, reason="ln after all exp")

            for r in range(RT):
                dh = dyn.tile([128, 1], f32, tag="dh", name="dh")
                nc.vector.tensor_tensor(dh[:], zh[r][:], lse[(r, "head")][:],
                                        OP.subtract)
                u1 = dyn.tile([128, 1], f32, tag="u1", name="u1")
                nc.vector.scalar_tensor_tensor(
                    u1[:], z1[r][:], lse[(r, "t1")][:], msk_t[r][:, 1:2],
                    OP.subtract, OP.mult,
                )
                acc1 = dyn.tile([128, 1], f32, tag="acc1", name="acc1")
                nc.vector.tensor_tensor(acc1[:], dh[:], u1[:], OP.add)
                if (r, "t0") in plans:
                    u0 = dyn.tile([128, 1], f32, tag="u0", name="u0")
                    nc.vector.scalar_tensor_tensor(
                        u0[:], z0[r][:], lse[(r, "t0")][:], msk_t[r][:, 0:1],
                        OP.subtract, OP.mult,
                    )
                    nc.vector.tensor_tensor(outsb[:, r:r + 1], acc1[:], u0[:],
                                            OP.add)
                else:
                    nc.vector.tensor_copy(outsb[:, r:r + 1], acc1[:])

            # ---------------- outputs + loss -------------------------------
            tp1 = pstile("tp1")
            nc.tensor.transpose(tp1[:RT, :128], outsb[:], ident_t[:])
            outT = w.tile([RT, 128], f32, name="outT", tag="outT")
            nc.vector.tensor_copy(outT[:], tp1[:RT, :128])
            nc.sync.dma_start(d_out[:].rearrange("(r p) -> r p", p=128),
                              outT[:])

            lsum = w.tile([RT, 1], f32, name="lsum", tag="lsum")
            nc.vector.tensor_reduce(lsum[:], outT[:], AX.X, OP.add)
            # cross-partition 4-sum via DRAM roundtrip reshape (K=4 PE
            # transposes are flaky on HW)
            lscr = dram.tile([RT, 1], f32, name="lscr")
            nc.sync.dma_start(lscr[:], lsum[:])
            lrow = w.tile([1, RT], f32, name="lrow", tag="lrow")
            nc.sync.dma_start(lrow[:], lscr[:].rearrange("r one -> one r"))
            lpart = w.tile([1, 1], f32, name="lpart", tag="lpart")
            nc.vector.tensor_reduce(lpart[:], lrow[:], AX.X, OP.add)

            if opts["use_collective"]:
                cc_in = dram.tile([1, 1], f32, name="cc_in")
                cc_out = dram.tile([1, 1], f32, name="cc_out",
                                   addr_space="Shared")
                nc.gpsimd.dma_start(cc_in[:], lpart[:])
                nc.gpsimd.collective_compute(
                    "AllReduce", OP.add,
                    replica_groups=[list(range(NC))],
                    ins=[cc_in[:].opt()], outs=[cc_out[:].opt()],
                )
                gl = w.tile([1, 1], f32, name="gl", tag="gl")
                nc.gpsimd.dma_start(gl[:], cc_out[:])
                nc.vector.tensor_scalar_mul(gl[:], gl[:], -1.0 / N)
                nc.sync.dma_start(d_loss[:, :], gl[:])
            else:
                nc.sync.dma_start(d_loss[:, :], lpart[:])

    nc.compile()
    return nc


# --------------------------------------------------------------------------
# entry points
# --------------------------------------------------------------------------

def run(inputs_dict, trace=False, opts=None):
    opts = {**_OPTS, **(opts or {})}
    in_maps, perm, t0_nrt = _host_prep(
        inputs_dict["inputs"], inputs_dict["targets"], inputs_dict["head_W"],
        inputs_dict["emb0"], inputs_dict["lin0"], inputs_dict["emb1"],
        inputs_dict["lin1"], opts,
    )
    nc = _build(t0_nrt, opts)
    res = run_bass_kernel_spmd(nc, in_maps, core_ids=list(range(NC)),
                               trace=trace)
    outs = np.concatenate([
        np.asarray(res.results[c]["out"], np.float32).reshape(-1)
        for c in range(NC)
    ])
    outputs = np.empty(N, np.float32)
    outputs[perm] = outs
    if opts["use_collective"]:
        loss = np.float32(np.asarray(res.results[0]["loss"]).reshape(-1)[0])
    else:
        parts = [np.asarray(res.results[c]["loss"]).reshape(-1)[0]
                 for c in range(NC)]
        loss = np.float32(-np.sum(parts) / N)
    return outputs, loss, res


def kernel(**inputs):
    outputs, loss, _ = run(inputs, trace=False)
    return outputs, loss


# revision 47
# speedup vs baseline: 1.0560x; 1.0560x over previous
"""Trainium2 Bass kernel for nn_AdaptiveSoftmax (8 NeuronCores, data-parallel).

Adaptive softmax loss: head [2002 vocab] + two low-rank tail clusters
(8000 @ rank-256, 40257 @ rank-64).  Per row: out = logprob at target,
loss = mean(-out).

Strategy (per core, 512 rows, bf16 matmul / f32 accumulation):
  - Host precomputes transposed, bf16, log2e-scaled weight layouts, the
    gathered target weight rows (so no on-device indexing), and cluster
    masks.  Rows are permuted so cluster-1 (tail0) rows land in row-tile 0
    of each core, collapsing tail0 work 4x.
  - Device computes all logits with TensorE (tail1's K=64 matmuls are
    row-packed two-at-a-time via tile_position) into a single [128,4096]
    PSUM ring (all 8 banks, Tile subtile deps), drained by two parallel
    exp lanes computing the log-sum-exp:
      ACT lane:  activation(Exp, scale=ln2, accum_out) in-place on PSUM,
                 double-buffered 1536-col regions
      DVE lane:  Schraudolph fast-exp2 (tensor_scalar f32->int32 convert,
                 bitcast f32, pair-batched reduce) on a 1024-col region --
                 ~3% elementwise error, zero-mean, so sums are good to 1e-3
  - Gathered logit z_t = <x, W[t]> via scalar_tensor_tensor accum dots.
  - out = (zh - lse_h) + m0*(z0 - lse0) + m1*(z1 - lse1)
  - per-core loss partials summed on the host during unshard (a device
    AllReduce of the scalar measured ~38us and was dropped).
"""

import sys

sys.path.insert(0, "/opt/trn_rl_repo")

import numpy as np
import ml_dtypes

import concourse.bass as bass
import concourse.tile as tile
from concourse import bacc, mybir
from concourse.bass_utils import run_bass_kernel_spmd

BF16 = ml_dtypes.bfloat16

# ---- problem constants (hardcoded; harness supplies matching shapes) ----
N = 4096
NDIM = 1024
NTOK = 50257
C0, C1 = 2000, 10000
HS = 2002  # head size
D0, D1 = 256, 64
SZ0 = C1 - C0          # 8000
SZ1 = NTOK - C1        # 40257
NC = 8                 # cores
R = N // NC            # 512 rows per core
RT = R // 128          # 4 row tiles
H1 = (SZ1 + 1) // 2    # 20129 stacked-half width
PADCOLS = 2 * H1 - SZ1 # 1 pad column in half B

LOG2E = float(np.log2(np.e))
LN2 = float(np.log(2.0))
SC23 = float(1 << 23)
MAGIC_C = 482583.0
MAGIC = float(127 * (1 << 23) - MAGIC_C)

_OPTS = dict(
    use_dve_lane=True,
    group_rows=True,
    chunk_cols=2048,      # psum chunk free size (multiple of 512 + remainder)
    psum_bufs=2,
    use_collective=False,
    act_cost=lambda n: (n + 690.0) / 1.2,               # ns per ACT chunk (meas)
    dve_cost=lambda n: 2.0 * (n + 150.0) / 0.96,        # ns per DVE chunk (meas)
)

f32 = mybir.dt.float32
bf16 = mybir.dt.bfloat16
i32 = mybir.dt.int32
AX = mybir.AxisListType
OP = mybir.AluOpType
AF = mybir.ActivationFunctionType


# --------------------------------------------------------------------------
# host-side prep
# --------------------------------------------------------------------------

def _host_prep(inputs, targets, head_W, emb0, lin0, emb1, lin1, opts):
    x = np.asarray(inputs, np.float32)
    t = np.asarray(targets).astype(np.int64)
    head_W = np.asarray(head_W, np.float32)
    emb0 = np.asarray(emb0, np.float32)
    lin0 = np.asarray(lin0, np.float32)
    emb1 = np.asarray(emb1, np.float32)
    lin1 = np.asarray(lin1, np.float32)

    cluster = np.zeros(N, np.int64)
    cluster[(t >= C0) & (t < C1)] = 1
    cluster[t >= C1] = 2

    if opts["group_rows"]:
        # deal rows of each cluster round-robin across cores, then order each
        # core's rows cluster1-first so tail0 rows pack into the first tiles
        core_rows = [[] for _ in range(NC)]
        for cl in (1, 2, 0):
            idx = np.nonzero(cluster == cl)[0]
            for j, row in enumerate(idx):
                core_rows[j % NC].append(row)
        order = []
        for c in range(NC):
            rows = core_rows[c]
            rows.sort(key=lambda r: (0 if cluster[r] == 1 else 1))
            order.extend(rows)
        perm = np.array(order, np.int64)
        n1_max = max(int((cluster[perm[c * R:(c + 1) * R]] == 1).sum())
                     for c in range(NC))
        t0_nrt = max(1, -(-n1_max // 128))
    else:
        perm = np.arange(N, dtype=np.int64)
        t0_nrt = RT

    xp = x[perm]
    tp = t[perm]
    head_targets = np.where(tp >= C1, C0 + 1, np.where(tp >= C0, C0, tp))
    rel0 = np.clip(tp - C0, 0, SZ0 - 1)
    rel1 = np.clip(tp - C1, 0, SZ1 - 1)
    m0 = ((tp >= C0) & (tp < C1)).astype(np.float32)
    m1 = (tp >= C1).astype(np.float32)

    if opts["group_rows"]:
        # every tail0 row must live in the first t0_nrt tiles of its core
        for c in range(NC):
            mm = m0[c * R:(c + 1) * R]
            assert mm[t0_nrt * 128:].sum() == 0, "tail0 row outside tail0 tiles"

    Wg = head_W[head_targets]          # [N,1024]
    e0g = emb0[rel0]                   # [N,256]
    e1g = emb1[rel1]                   # [N,64]

    WT = np.ascontiguousarray(head_W.T) * LOG2E        # [1024,2002]
    e0T = np.ascontiguousarray(emb0.T) * LOG2E         # [256,8000]
    e1T = np.ascontiguousarray(emb1.T) * LOG2E         # [64,40257]
    E1 = np.zeros((128, H1), np.float32)
    E1[:64] = e1T[:, :H1]
    E1[64:, : SZ1 - H1] = e1T[:, H1:]

    def b(a):
        return np.ascontiguousarray(a).astype(BF16)

    def cols(a3):
        # [G,128,F] -> [128, G*F] keeping each [128,F] block column-contig
        return np.concatenate([a3[g] for g in range(a3.shape[0])], axis=1)

    # consolidated column-packed blocks (one DMA each, sliced on SBUF side)
    pk_wt = np.concatenate(
        [cols(WT.reshape(8, 128, HS)), cols(lin0.reshape(8, 128, D0))], axis=1)
    pk_e0 = cols(e0T.reshape(2, 128, SZ0))
    e1s = b(E1)
    ident = np.eye(128, dtype=np.float32)

    in_maps = []
    for c in range(NC):
        s = slice(c * R, (c + 1) * R)
        xc = xp[s]
        xT = np.ascontiguousarray(xc.T)            # [1024,512]
        pk_x = np.concatenate(
            [cols(xT.reshape(8, 128, R)), cols(lin1.reshape(8, 128, D1))],
            axis=1)
        pk_g = np.concatenate(
            [cols(xc.reshape(RT, 128, NDIM)),
             cols(Wg[s].reshape(RT, 128, NDIM)),
             cols(e0g[s].reshape(RT, 128, D0)),
             cols(e1g[s].reshape(RT, 128, D1))], axis=1)
        in_maps.append({
            "pkx": b(pk_x),             # xt 8x512 | lin1 8x64
            "e1s": e1s,                 # stacked tail1 (log2e-scaled)
            "pkw": b(pk_wt),            # wt 8x2002 | lin0 8x256  (replicated)
            "pke0": b(pk_e0),           # e0t 2x8000              (replicated)
            "pkg": b(pk_g),             # x4 | wg | e0g | e1g  (4x each)
            "msk": np.ascontiguousarray(
                np.stack([m0[s], m1[s]], axis=-1).reshape(RT, 128, 2)
            ).astype(np.float32),
            "ident": ident,
        })
    return in_maps, perm, t0_nrt


# --------------------------------------------------------------------------
# chunk planning
# --------------------------------------------------------------------------

def _plan_cluster_chunks(mms, chunk_cols):
    """Group MM specs into psum chunks.  HW constraint: every matmul output
    must START on a PSUM bank boundary (multiple of 512 f32), so slots are
    512-aligned; a partial-width MM (n<512) ends its chunk so the chunk's
    written region stays contiguous from offset 0."""
    chunks = []
    cur, off = [], 0
    for mm in mms:
        n = mm["n"]
        if off + n > chunk_cols:
            chunks.append(cur)
            cur, off = [], 0
        cur.append((off, mm))
        if n < 512:
            chunks.append(cur)
            cur, off = [], 0
        else:
            off += 512
    if cur:
        chunks.append(cur)
    return chunks


def _col_splits(total, step=512):
    out = []
    c = 0
    while c < total:
        n = min(step, total - c)
        out.append((c, n))
        c += n
    return out


# --------------------------------------------------------------------------
# device program
# --------------------------------------------------------------------------

def _build(t0_nrt, opts):
    nc = bacc.Bacc("TRN2", target_bir_lowering=False, debug=False, num_devices=NC)

    PKX = 8 * R + 8 * D1
    PKW = 8 * HS + 8 * D0
    PKE0 = 2 * SZ0
    PKG = RT * (NDIM + NDIM + D0 + D1)
    d_pkx = nc.dram_tensor("pkx", [128, PKX], bf16, kind="ExternalInput")
    d_e1s = nc.dram_tensor("e1s", [128, H1], bf16, kind="ExternalInput")
    d_pkw = nc.dram_tensor("pkw", [128, PKW], bf16, kind="ExternalInput")
    d_pke0 = nc.dram_tensor("pke0", [128, PKE0], bf16, kind="ExternalInput")
    d_pkg = nc.dram_tensor("pkg", [128, PKG], bf16, kind="ExternalInput")
    d_msk = nc.dram_tensor("msk", [RT, 128, 2], f32, kind="ExternalInput")
    d_ident = nc.dram_tensor("ident", [128, 128], f32, kind="ExternalInput")

    d_out = nc.dram_tensor("out", [R], f32, kind="ExternalOutput")
    d_loss = nc.dram_tensor("loss", [RT, 1], f32, kind="ExternalOutput")

    CH = opts["chunk_cols"]
    act_cost, dve_cost = opts["act_cost"], opts["dve_cost"]

    with tile.TileContext(nc) as tc:
        with (
            tc.tile_pool(name="w", bufs=1) as w,        # persistent
            tc.tile_pool(name="dyn", bufs=3) as dyn,    # rotating scratch
            tc.tile_pool(name="ps", bufs=1, space="PSUM") as ps,
            tc.tile_pool(name="dram", bufs=1, space="DRAM") as dram,
        ):
            # ONE psum tile spanning all 8 banks; every PSUM user takes
            # 512-aligned regions of it (Tile subtile deps handle sync)
            ring = ps.tile([128, 4096], f32, tag="ring", name="ring")
            _pj = [0]

            def pstile(name):
                # projection scratch: alternate two 512-col ring regions
                base = (_pj[0] % 2) * 512
                _pj[0] += 1
                return ring[:, base: base + 512]
            # ---------------- input DMA (consolidated, need-ordered) -------
            pkx_t = w.tile([128, PKX], bf16, name="pkx", tag="pkx")
            nc.sync.dma_start(pkx_t[:], d_pkx[:, :])
            xt_t = [pkx_t[:, k * R:(k + 1) * R] for k in range(8)]
            lin1_t = [pkx_t[:, 8 * R + k * D1: 8 * R + (k + 1) * D1]
                      for k in range(8)]

            e1_t = w.tile([128, H1], bf16, name="e1t", tag="e1t")
            E1SPLIT = 4
            _e1step = ((H1 + E1SPLIT - 1) // E1SPLIT + 511) // 512 * 512
            for j in range(0, H1, _e1step):
                jn = min(_e1step, H1 - j)
                nc.sync.dma_start(e1_t[:, j: j + jn], d_e1s[:, j: j + jn])

            pkw_t = w.tile([128, PKW], bf16, name="pkw", tag="pkw")
            nc.sync.dma_start(pkw_t[:], d_pkw[:, :])
            wt_t = [pkw_t[:, k * HS:(k + 1) * HS] for k in range(8)]
            lin0_t = [pkw_t[:, 8 * HS + k * D0: 8 * HS + (k + 1) * D0]
                      for k in range(8)]

            pke0_t = w.tile([128, PKE0], bf16, name="pke0", tag="pke0")
            nc.sync.dma_start(pke0_t[:], d_pke0[:, :])
            e0t_t = [pke0_t[:, k * SZ0:(k + 1) * SZ0] for k in range(2)]

            pkg_t = w.tile([128, PKG], bf16, name="pkg", tag="pkg")
            nc.sync.dma_start(pkg_t[:], d_pkg[:, :])
            _o = 0
            x_t = [pkg_t[:, _o + r * NDIM: _o + (r + 1) * NDIM]
                   for r in range(RT)]
            _o += RT * NDIM
            wg_t = [pkg_t[:, _o + r * NDIM: _o + (r + 1) * NDIM]
                    for r in range(RT)]
            _o += RT * NDIM
            e0g_t = [pkg_t[:, _o + r * D0: _o + (r + 1) * D0]
                     for r in range(RT)]
            _o += RT * D0
            e1g_t = [pkg_t[:, _o + r * D1: _o + (r + 1) * D1]
                     for r in range(RT)]
            msk_t = [w.tile([128, 2], f32, name=f"msk{r}", tag=f"msk{r}")
                     for r in range(RT)]
            for r in range(RT):
                nc.sync.dma_start(msk_t[r][:], d_msk[r])
            ident_t = w.tile([128, 128], f32, name="ident", tag="ident")
            nc.sync.dma_start(ident_t[:], d_ident[:, :])

            # ---------------- projections ---------------------------------
            # only h1T (which gates tail1) runs up-front; h0T and the per-rt
            # h0/h1 z-dot projections are woven into the main loop as thunks
            h1T_sb = w.tile([128, R], bf16, name="h1T", tag="h1T")
            pt = pstile("pj1")
            for h in range(2):
                for k in range(8):
                    nc.tensor.matmul(
                        pt[h * 64:(h + 1) * 64, :R], lin1_t[k][:], xt_t[k][:],
                        start=(k == 0), stop=(k == 7),
                        tile_position=(0, h * 64),
                    )
            nc.vector.tensor_copy(h1T_sb[:], pt[:, :R])

            h0T_sb = [w.tile([128, R], bf16, name=f"h0T{m}", tag=f"h0T{m}")
                      for m in range(2)]
            h0_sb = [w.tile([128, D0], bf16, name=f"h0_{r}", tag=f"h0_{r}")
                     for r in range(RT)]
            h1_sb = [w.tile([128, D1], bf16, name=f"h1_{r}", tag=f"h1_{r}")
                     for r in range(RT)]
            zh = [w.tile([128, 1], f32, name=f"zh{r}", tag=f"zh{r}")
                  for r in range(RT)]
            z0 = [w.tile([128, 1], f32, name=f"z0{r}", tag=f"z0{r}")
                  for r in range(RT)]
            z1 = [w.tile([128, 1], f32, name=f"z1{r}", tag=f"z1{r}")
                  for r in range(RT)]

            # ---------------- chunk plans ----------------------------------
            # MM spec: dict(kind, n, ...); kinds: head, t0, t1
            def head_mms(r):
                return [dict(kind="head", r=r, c0=c, n=n)
                        for c, n in _col_splits(HS)]

            def t0_mms(r):
                return [dict(kind="t0", r=r, c0=c, n=n)
                        for c, n in _col_splits(SZ0)]

            def t1_mms(r):
                out = []
                for c, n in _col_splits(H1):
                    out.append(dict(kind="t1", r=r, h=0, c0=c, n=n))
                    out.append(dict(kind="t1", r=r, h=1, c0=c, n=n))
                return out

            # ------------- unit planning (ring regions + lanes) -------------
            # PSUM = one [128,4096] ring tile (all 8 banks).  Fixed regions:
            # A0=[0:1536], A1=[1536:3072] double-buffer the ACT lane;
            # D0=[3072:4096] feeds the DVE lane, whose q-reduce overlaps the
            # next fill.  Tile's subtile deps give region-level sync.
            A_COLS, D_COLS = 1536, 1024

            def plan_units(mms, lane_time, is_head, act_after=1.0):
                units = []
                i = 0
                while i < len(mms):
                    if not opts["use_dve_lane"] or i >= act_after * len(mms):
                        # tail of the emission stream: DVE still owes flush
                        # reduces + the combine chain there, so keep it free
                        lane = "act"
                    else:
                        lane = ("act" if lane_time["act"] <= lane_time["dve"]
                                else "dve")
                    want = 512 if is_head else (A_COLS if lane == "act"
                                                else D_COLS)
                    group, used, slots = [], 0, 0
                    while i < len(mms) and slots * 512 + mms[i]["n"] <= want:
                        mm = mms[i]
                        group.append((slots * 512, mm))
                        used = slots * 512 + mm["n"]
                        slots += 1
                        i += 1
                        if mm["n"] < 512:
                            break
                    if lane == "act":
                        lane_time["act"] += act_cost(used)
                    else:
                        lane_time["dve"] += dve_cost(used)
                    units.append(dict(lane=lane, group=group, used=used))
                return units

            lane_time = {"act": 0.0, "dve": 0.0}
            plans = {}
            for r in range(RT):
                t1_act_after = 0.6 if r == RT - 1 else 1.0
                plans[(r, "t1")] = plan_units(t1_mms(r), lane_time, False,
                                              act_after=t1_act_after)
                plans[(r, "head")] = plan_units(head_mms(r), lane_time, True)
                if r < t0_nrt:
                    # t0 units weave at the very end of the stream where the
                    # DVE lane is the long pole -- keep them on ACT
                    plans[(r, "t0")] = plan_units(t0_mms(r), lane_time, False,
                                                  act_after=0.0)

            # emission order: tail1 from the start; everything else woven in
            t1_seq = [((r, "t1"), ci) for r in range(RT)
                      for ci in range(len(plans[(r, "t1")]))]
            extras = [("h0T", 0), ("h0T", 1)]
            extras += [((r, "head"), ci) for r in range(RT)
                       for ci in range(len(plans[(r, "head")]))]
            extras += [((r, "t0"), ci) for r in range(t0_nrt)
                       for ci in range(len(plans[(r, "t0")]))]
            extras += [("hz", r) for r in range(RT)]
            # extras need their weights: pkw ~30us (~pos 16), pke0 ~41us,
            # pkg ~49us.  Weave every 2nd t1 unit from pos 16, then every
            # unit so all extras land before the t1 stream ends.
            npos = len(t1_seq)
            positions = list(range(16, min(48, npos), 2))
            positions += list(range(max(positions[-1] + 1, 48) if positions
                                    else 48, npos))
            pos_iter = iter(positions)
            nxt = next(pos_iter, None)
            seq = []
            ei = 0
            for idx, item in enumerate(t1_seq):
                seq.append(item)
                while nxt is not None and idx == nxt and ei < len(extras):
                    seq.append(extras[ei])
                    ei += 1
                    nxt = next(pos_iter, None)
            seq.extend(extras[ei:])

            # accumulator slot count: one per act unit + one per dve flush.
            # Only ONE dve q-group may be open at a time (q-ring reuse), so
            # a key change flushes the previous owner's group -> replay seq.
            nslots = {key: 0 for key in plans}
            _owner, _pend = [None], [0]
            for key, ci in seq:
                if key in ("h0T", "hz"):
                    continue
                u = plans[key][ci]
                if u["lane"] == "act":
                    nslots[key] += 1
                else:
                    if _owner[0] is not None and _owner[0] != key and _pend[0]:
                        nslots[_owner[0]] += 1
                        _pend[0] = 0
                    _owner[0] = key
                    _pend[0] += 1
                    if _pend[0] == 2:
                        nslots[key] += 1
                        _pend[0] = 0
            if _pend[0]:
                nslots[_owner[0]] += 1
            sums = {}
            for key in plans:
                r, cl = key
                sums[key] = w.tile([128, max(nslots[key], 1)], f32,
                                   name=f"s_{cl}{r}", tag=f"s_{cl}{r}")

            qring = w.tile([128, 4096], i32, name="qring", tag="qring")

            state = dict(a_flip=0, q_flip=0, acc_ci={k: 0 for k in plans},
                         pend={k: [] for k in plans}, owner=None)

            def emit_mms(base, group, r):
                for off, mm in group:
                    dst = ring[:, base + off: base + off + mm["n"]]
                    if mm["kind"] == "head":
                        for k in range(8):
                            nc.tensor.matmul(
                                dst, xt_t[k][:, r * 128:(r + 1) * 128],
                                wt_t[k][:, mm["c0"]: mm["c0"] + mm["n"]],
                                start=(k == 0), stop=(k == 7),
                            )
                    elif mm["kind"] == "t0":
                        for k in range(2):
                            nc.tensor.matmul(
                                dst, h0T_sb[k][:, r * 128:(r + 1) * 128],
                                e0t_t[k][:, mm["c0"]: mm["c0"] + mm["n"]],
                                start=(k == 0), stop=(k == 1),
                            )
                    else:
                        h = mm["h"]
                        nc.tensor.matmul(
                            dst,
                            h1T_sb[h * 64:(h + 1) * 64, r * 128:(r + 1) * 128],
                            e1_t[h * 64:(h + 1) * 64,
                                 mm["c0"]: mm["c0"] + mm["n"]],
                            start=True, stop=True,
                            tile_position=(h * 64, 0),
                        )

            def flush_dve(key):
                pend = state["pend"][key]
                if not pend:
                    return
                base = pend[0][0]
                total = sum(p[1] for p in pend)
                ci = state["acc_ci"][key]
                nc.vector.tensor_reduce(
                    sums[key][:, ci: ci + 1],
                    qring[:, base: base + total].bitcast(f32),
                    AX.X, OP.add,
                )
                state["acc_ci"][key] = ci + 1
                state["pend"][key] = []

            # pad column in half B contributes exp(0)'s lane-dependent value
            # to every t1 sum (ACT: 1.0, DVE: bitcast(round(MAGIC)))
            dve_pad = float(
                np.array([int(round(MAGIC))], np.int32).view(np.float32)[0]
            )
            stot = {}
            for key in plans:
                r, cl = key
                stot[key] = w.tile([128, 1], f32, name=f"st_{cl}{r}",
                                   tag=f"st_{cl}{r}")
            remaining = {key: len(plans[key]) for key in plans}

            def key_done(key):
                # eager per-key total: flush leftovers + reduce sums + pad fix
                if state["owner"] == key:
                    flush_dve(key)
                    state["owner"] = None
                r, cl = key
                nc.vector.tensor_reduce(stot[key][:], sums[key][:], AX.X,
                                        OP.add)
                if cl == "t1":
                    pad_lane = plans[key][-1]["lane"]
                    pad = (1.0 if pad_lane == "act" else dve_pad) * PADCOLS
                    nc.vector.tensor_scalar_add(stot[key][:], stot[key][:],
                                                -pad)

            def emit_unit(key, unit):
                r, cl = key
                used = unit["used"]
                if unit["lane"] == "act":
                    base = state["a_flip"] * A_COLS
                    state["a_flip"] ^= 1
                    emit_mms(base, unit["group"], r)
                    src = ring[:, base: base + used]
                    ci = state["acc_ci"][key]
                    state["acc_ci"][key] = ci + 1
                    state["last_exp"] = nc.scalar.activation(
                        src, src, AF.Exp, scale=LN2,
                        accum_out=sums[key][:, ci: ci + 1])
                else:
                    base = 3072
                    emit_mms(base, unit["group"], r)
                    if state["owner"] is not None and state["owner"] != key:
                        flush_dve(state["owner"])
                    state["owner"] = key
                    pend = state["pend"][key]
                    if not pend:
                        qbase = state["q_flip"] * 2048
                        state["q_flip"] ^= 1
                    else:
                        qbase = pend[0][0] + pend[0][1]
                    nc.vector.tensor_scalar(
                        qring[:, qbase: qbase + used],
                        ring[:, base: base + used],
                        SC23, MAGIC, OP.mult, OP.add,
                    )
                    pend.append((qbase, used))
                    if len(pend) == 2:
                        flush_dve(key)
                remaining[key] -= 1
                if remaining[key] == 0:
                    key_done(key)

            def emit_h0T(m):
                base = state["a_flip"] * A_COLS
                state["a_flip"] ^= 1
                pt = ring[:, base: base + R]
                for k in range(8):
                    nc.tensor.matmul(
                        pt, lin0_t[k][:, m * 128:(m + 1) * 128],
                        xt_t[k][:], start=(k == 0), stop=(k == 7),
                    )
                nc.vector.tensor_copy(h0T_sb[m][:], pt)

            def emit_hz(r):
                # h0/h1 projections + the three gathered-logit dots for rt r
                base = state["a_flip"] * A_COLS
                state["a_flip"] ^= 1
                pt = ring[:, base: base + D0]
                for k in range(8):
                    nc.tensor.matmul(
                        pt, xt_t[k][:, r * 128:(r + 1) * 128],
                        lin0_t[k][:], start=(k == 0), stop=(k == 7),
                    )
                nc.vector.tensor_copy(h0_sb[r][:], pt)
                pt2 = ring[:, base + 512: base + 512 + D1]
                for k in range(8):
                    nc.tensor.matmul(
                        pt2, xt_t[k][:, r * 128:(r + 1) * 128],
                        lin1_t[k][:], start=(k == 0), stop=(k == 7),
                    )
                nc.vector.tensor_copy(h1_sb[r][:], pt2)
                s = dyn.tile([128, NDIM], bf16, tag="ttr", name="ttr")
                nc.vector.scalar_tensor_tensor(
                    s[:, :NDIM], x_t[r][:], 1.0, wg_t[r][:],
                    OP.mult, OP.mult, accum_out=zh[r][:],
                )
                s = dyn.tile([128, NDIM], bf16, tag="ttr", name="ttr")
                nc.vector.scalar_tensor_tensor(
                    s[:, :D0], h0_sb[r][:], 1.0, e0g_t[r][:],
                    OP.mult, OP.mult, accum_out=z0[r][:],
                )
                s = dyn.tile([128, NDIM], bf16, tag="ttr", name="ttr")
                nc.vector.scalar_tensor_tensor(
                    s[:, :D1], h1_sb[r][:], 1.0, e1g_t[r][:],
                    OP.mult, OP.mult, accum_out=z1[r][:],
                )

            for key, ci in seq:
                if key == "h0T":
                    emit_h0T(ci)
                elif key == "hz":
                    emit_hz(ci)
                else:
                    emit_unit(key, plans[key][ci])

            # ------------- epilogue: ln + combine + outputs ----------------
            outsb = w.tile([128, RT], f32, name="outsb", tag="outsb")
            lse = {}
            for key in plans:
                r, cl = key
                lse[key] = w.tile([128, 1], f32, name=f"lse_{cl}{r}",
                                  tag=f"lse_{cl}{r}")
            # pin the Ln block after the last Exp: eager stot frees Ln deps
            # mid-run and the scheduler would interleave Ln into ACT idle
            # gaps, paying a ~2.6us ACT table-set round-trip each time
            for key in plans:
                ln = nc.scalar.activation(lse[key][:], stot[key][:], AF.Ln)
                if state.get("last_exp") is not None:
                    tile.add_dep_helper(ln.ins, state["last_exp"].ins,
                                        sync=True, reason="ln after all exp")

            for r in range(RT):
                dh = dyn.tile([128, 1], f32, tag="dh", name="dh")
                nc.vector.tensor_tensor(dh[:], zh[r][:], lse[(r, "head")][:],
                                        OP.subtract)
                u1 = dyn.tile([128, 1], f32, tag="u1", name="u1")
                nc.vector.scalar_tensor_tensor(
                    u1[:], z1[r][:], lse[(r, "t1")][:], msk_t[r][:, 1:2],
                    OP.subtract, OP.mult,
                )
                acc1 = dyn.tile([128, 1], f32, tag="acc1", name="acc1")
                nc.vector.tensor_tensor(acc1[:], dh[:], u1[:], OP.add)
                if (r, "t0") in plans:
                    u0 = dyn.tile([128, 1], f32, tag="u0", name="u0")
                    nc.vector.scalar_tensor_tensor(
                        u0[:], z0[r][:], lse[(r, "t0")][:], msk_t[r][:, 0:1],
                        OP.subtract, OP.mult,
                    )
                    nc.vector.tensor_tensor(outsb[:, r:r + 1], acc1[:], u0[:],
                                            OP.add)
                else:
                    nc.vector.tensor_copy(outsb[:, r:r + 1], acc1[:])

            # ---------------- outputs + loss -------------------------------
            tp1 = pstile("tp1")
            nc.tensor.transpose(tp1[:RT, :128], outsb[:], ident_t[:])
            outT = w.tile([RT, 128], f32, name="outT", tag="outT")
            nc.vector.tensor_copy(outT[:], tp1[:RT, :128])
            nc.sync.dma_start(d_out[:].rearrange("(r p) -> r p", p=128),
                              outT[:])

            # per-row-tile loss partials; the host sums 4*NC values during
            # unshard (a DRAM-roundtrip 4-way partition sum cost ~4us of
            # serial tail, and a device AllReduce measured ~38us)
            lsum = w.tile([RT, 1], f32, name="lsum", tag="lsum")
            nc.vector.tensor_reduce(lsum[:], outT[:], AX.X, OP.add)
            nc.sync.dma_start(d_loss[:, :], lsum[:])

    nc.compile()
    return nc


# --------------------------------------------------------------------------
# entry points
# --------------------------------------------------------------------------

def run(inputs_dict, trace=False, opts=None):
    opts = {**_OPTS, **(opts or {})}
    in_maps, perm, t0_nrt = _host_prep(
        inputs_dict["inputs"], inputs_dict["targets"], inputs_dict["head_W"],
        inputs_dict["emb0"], inputs_dict["lin0"], inputs_dict["emb1"],
        inputs_dict["lin1"], opts,
    )
    nc = _build(t0_nrt, opts)
    res = run_bass_kernel_spmd(nc, in_maps, core_ids=list(range(NC)),
                               trace=trace)
    outs = np.concatenate([
        np.asarray(res.results[c]["out"], np.float32).reshape(-1)
        for c in range(NC)
    ])
    outputs = np.empty(N, np.float32)
    outputs[perm] = outs
    parts = [np.asarray(res.results[c]["loss"], np.float32).reshape(-1)
             for c in range(NC)]
    loss = np.float32(-np.concatenate(parts).sum() / N)
    return outputs, loss, res


def kernel(**inputs):
    outputs, loss, _ = run(inputs, trace=False)
    return outputs, loss
